# revision 10
# baseline (speedup 1.0000x reference)
"""Trainium2 Bass kernel for nn_CombinedModel_15977278341388 (nms_detection).

Sharding (8 NeuronCores, memory-bound):
  * wr2 layer-1 matmul (65536x100, 26MB): contraction sharded by channel;
    (100,B) partials AllReduce'd on-device (3.2KB).
  * wr2 layers 2/3 + postfix folded on host into one (100,4) matmul (weight
    preprocessing only -- no input data touched on host).
  * ROI adaptive max pool: channel-sharded (each core pools its 8 channels of
    all samples / all 3 pyramid levels).  Row windows are fetched straight
    from HBM with dma_gather (per-(c,i) row blocks), trailing garbage rows
    masked via tensor_mask_reduce's select-to -FLT_MAX, then a static
    tensor_reduce max; column bins via tensor_mask_reduce windows whose
    per-(b,j) bounds are computed on-device from the predicted boxes.
  * head_w1 (7x24576x128, 88MB): contraction-dim sharded to exactly the
    pooled channels each core owns -> per-core (128hcol, 7*8) partials,
    one 28KB AllReduce, then ReLU + 7 small output matmuls (replicated).

kernel(**inputs) takes FULL inputs, shards internally, returns the FULL
7-tuple (o1, o2, ad0..ad4).
"""
import numpy as np
from contextlib import ExitStack

import concourse.bass as bass
import concourse.mybir as mybir
import concourse.tile as tile
from concourse import bacc
from concourse.bass_utils import run_bass_kernel_spmd

F32 = mybir.dt.float32
I16 = mybir.dt.int16
I32 = mybir.dt.int32
ALU = mybir.AluOpType
ACTF = mybir.ActivationFunctionType
AXX = mybir.AxisListType

P = 128
B = 8
NCORES = 8
CPC = 8            # channels per core
POOL_H, POOL_W = 16, 8
NEGF = float(np.finfo(np.float32).min)

# (H, W, gather_rows, row_unit).  x3 uses 2-row units so the dma_gather
# stride is 256B (the HW minimum); a 4-row window + parity mask covers any
# 3-row bin window.
LVLS = [(128, 128, 9, 1), (64, 64, 5, 1), (32, 32, 4, 2)]
PADROWS = 16
D_HEADS = 7 * 128  # 896


def _floor_inplace(nc, spool, ap, tag):
    """ap <- floor(ap) for non-negative f32 values (|x| < 2^31).

    f32->i32->f32 round trip gives either trunc(x) or rne(x) depending on
    the cast mode; both land in {floor(x), floor(x)+1}, so subtracting the
    (rounded > x) flag yields an exact floor under either semantics.
    """
    ti = spool.tile(list(ap.shape), I32, tag=tag + "i")
    nc.vector.tensor_copy(out=ti[:], in_=ap)
    tf = spool.tile(list(ap.shape), F32, tag=tag + "f")
    nc.vector.tensor_copy(out=tf[:], in_=ti[:])
    gt = spool.tile(list(ap.shape), F32, tag=tag + "g")
    nc.vector.tensor_tensor(out=gt[:], in0=tf[:], in1=ap, op=ALU.is_gt)
    nc.vector.tensor_tensor(out=ap, in0=tf[:], in1=gt[:], op=ALU.subtract)


def _build(nc):
    # ---------------- DRAM I/O ----------------
    xs_dram = [
        nc.dram_tensor(f"xs{li}", [B * CPC * H + PADROWS, W], F32,
                       kind="ExternalInput")
        for li, (H, W, TG, RU) in enumerate(LVLS)
    ]
    x3t_d = nc.dram_tensor("x3t", [P, 64 * B], F32, kind="ExternalInput")
    w1p_d = nc.dram_tensor("w1p", [P, 64 * 100], F32, kind="ExternalInput")
    whp_d = nc.dram_tensor("whp", [P, 24 * D_HEADS], F32, kind="ExternalInput")
    wf_d = nc.dram_tensor("wf", [100, 4], F32, kind="ExternalInput")
    bf_d = nc.dram_tensor("bf", [4, 8], F32, kind="ExternalInput")
    scl_d = nc.dram_tensor("scl", [1, 96], F32, kind="ExternalInput")
    b1t_d = nc.dram_tensor("b1t", [P, 7], F32, kind="ExternalInput")
    w2c_d = nc.dram_tensor("w2c", [P, 238], F32, kind="ExternalInput")
    b2r_d = nc.dram_tensor("b2r", [8, 238], F32, kind="ExternalInput")
    out_d = nc.dram_tensor("out_all", [8, 238], F32, kind="ExternalOutput")

    with tile.TileContext(nc) as tc, ExitStack() as ctx:
        pool = ctx.enter_context(tc.tile_pool(name="main", bufs=1))
        gpool = ctx.enter_context(tc.tile_pool(name="gath", bufs=3))
        spool = ctx.enter_context(tc.tile_pool(name="scr", bufs=2))
        hpsp = ctx.enter_context(tc.tile_pool(name="hps", bufs=1, space="PSUM"))
        sps = ctx.enter_context(tc.tile_pool(name="sps", bufs=1, space="PSUM"))
        dram = ctx.enter_context(tc.tile_pool(name="dram", bufs=1, space="DRAM"))

        # ------------- head weights: stream in early, used late -----------
        whp = pool.tile([P, 24, D_HEADS], F32)
        for chunk in range(24):
            nc.sync.dma_start(
                whp[:, chunk], whp_d[:, chunk * D_HEADS:(chunk + 1) * D_HEADS])

        # ------------- stage A: h1 partial = x3_shard @ W1_shard ----------
        x3t = pool.tile([P, 64, B], F32)
        nc.sync.dma_start(x3t[:], x3t_d[:].rearrange("p (c b) -> p c b", b=B))
        w1p = pool.tile([P, 64, 100], F32)
        nc.sync.dma_start(w1p[:], w1p_d[:].rearrange("p (c o) -> p c o", o=100))
        h1ps = sps.tile([100, B], F32, tag="sps", name="h1ps")
        for ch in range(64):
            nc.tensor.matmul(out=h1ps[:], lhsT=w1p[:, ch], rhs=x3t[:, ch],
                             start=(ch == 0), stop=(ch == 63))
        h1sb = pool.tile([100, B], F32)
        nc.any.tensor_copy(out=h1sb[:], in_=h1ps[:])

        cc_in1 = dram.tile([100, B], F32)
        cc_out1 = dram.tile([100, B], F32)
        nc.sync.dma_start(cc_in1[:], h1sb[:])
        nc.gpsimd.collective_compute(
            "AllReduce", ALU.add, replica_groups=[list(range(NCORES))],
            ins=[cc_in1.opt()], outs=[cc_out1.opt()])
        h1t = pool.tile([100, B], F32)
        nc.sync.dma_start(h1t[:], cc_out1[:])

        # ------------- boxes: boxT = clip(h1 @ Wf + bf, 0, 1) -------------
        wf = pool.tile([100, 4], F32)
        nc.sync.dma_start(wf[:], wf_d[:])
        bfr = pool.tile([4, 8], F32)
        nc.sync.dma_start(bfr[:], bf_d[:])
        boxps = sps.tile([4, B], F32, tag="sps", name="boxps")
        nc.tensor.matmul(out=boxps[:], lhsT=wf[:], rhs=h1t[:],
                         start=True, stop=True)
        boxT = pool.tile([4, B], F32)
        nc.vector.tensor_add(out=boxT[:], in0=boxps[:], in1=bfr[:])
        nc.vector.tensor_scalar(out=boxT[:], in0=boxT[:], scalar1=0.0,
                                scalar2=1.0, op0=ALU.max, op1=ALU.min)

        # cRow (1,32) = [x1(8) | y1(8) | x2(8) | y2(8)] on partition 0
        cRow = pool.tile([1, 32], F32)
        nc.sync.dma_start(cRow[:].rearrange("p (c b) -> p c b", b=B), boxT[:])

        # ------------- integer coords for the 3 levels --------------------
        scl = pool.tile([1, 96], F32)      # [lvl, coord, b] = W/H/W/H
        nc.sync.dma_start(scl[:], scl_d[:])
        cAll = pool.tile([1, 96], F32)
        nc.vector.tensor_tensor(
            out=cAll[:].rearrange("p (l c) -> p l c", l=3),
            in0=cRow[:].unsqueeze(1).to_broadcast([1, 3, 32]),
            in1=scl[:].rearrange("p (l c) -> p l c", l=3), op=ALU.mult)
        _floor_inplace(nc, spool, cAll[:], "fr96")

        def crd(lvl, c):  # (1, 8) coordinate row of a level
            return cAll[:, lvl * 32 + c * 8: lvl * 32 + c * 8 + 8]

        # lenRow (1,48): [lvl][leny(8) | lenx(8)];  validRow (1,24): [lvl, b]
        lenRow = pool.tile([1, 48], F32)
        validRow = pool.tile([1, 24], F32)
        for lvl, (H, W, TG, RU) in enumerate(LVLS):
            ly = lenRow[:, lvl * 16: lvl * 16 + 8]
            lx = lenRow[:, lvl * 16 + 8: lvl * 16 + 16]
            nc.vector.tensor_tensor(out=ly, in0=crd(lvl, 3), in1=crd(lvl, 1),
                                    op=ALU.subtract)
            nc.vector.tensor_scalar(out=ly, in0=ly, scalar1=1.0, scalar2=1.0,
                                    op0=ALU.add, op1=ALU.max)
            nc.vector.tensor_tensor(out=lx, in0=crd(lvl, 2), in1=crd(lvl, 0),
                                    op=ALU.subtract)
            nc.vector.tensor_scalar(out=lx, in0=lx, scalar1=1.0, scalar2=1.0,
                                    op0=ALU.add, op1=ALU.max)
            vr = validRow[:, lvl * 8: lvl * 8 + 8]
            nc.vector.tensor_scalar(out=vr, in0=crd(lvl, 2), scalar1=float(W),
                                    scalar2=None, op0=ALU.is_lt)
            for in0, in1, op in ((crd(lvl, 3), float(H), ALU.is_lt),
                                 (crd(lvl, 2), crd(lvl, 0), ALU.is_gt),
                                 (crd(lvl, 3), crd(lvl, 1), ALU.is_gt)):
                vv = spool.tile([1, 8], F32, tag="vv")
                if isinstance(in1, float):
                    nc.vector.tensor_scalar(out=vv[:], in0=in0, scalar1=in1,
                                            scalar2=None, op0=op)
                else:
                    nc.vector.tensor_tensor(out=vv[:], in0=in0, in1=in1, op=op)
                nc.vector.tensor_tensor(out=vr, in0=vr, in1=vv[:], op=ALU.mult)

        # ------------- W-stage bounds rows (1, 192) -----------------------
        # col bin j of (lvl, b): [x1c + floor(j*lenx/8),
        #                         x1c + floor((j*lenx + lenx + 7)/8))
        jio = pool.tile([1, 192], I32)
        nc.gpsimd.iota(jio[:], pattern=[[0, 24], [1, 8]], base=0,
                       channel_multiplier=0)
        jiof = pool.tile([1, 192], F32)
        nc.any.tensor_copy(out=jiof[:], in_=jio[:])
        lenxB = pool.tile([1, 192], F32)
        x1cB = pool.tile([1, 192], F32)
        for lvl in range(3):
            nc.any.tensor_copy(
                out=lenxB[:, lvl * 64:(lvl + 1) * 64].rearrange(
                    "p (b j) -> p b j", j=8),
                in_=lenRow[:, lvl * 16 + 8: lvl * 16 + 16]
                .unsqueeze(2).to_broadcast([1, 8, 8]))
            nc.any.tensor_copy(
                out=x1cB[:, lvl * 64:(lvl + 1) * 64].rearrange(
                    "p (b j) -> p b j", j=8),
                in_=crd(lvl, 0).unsqueeze(2).to_broadcast([1, 8, 8]))
        tj = pool.tile([1, 192], F32)
        nc.vector.tensor_tensor(out=tj[:], in0=jiof[:], in1=lenxB[:],
                                op=ALU.mult)
        swRow = pool.tile([1, 192], F32)
        ewRow = pool.tile([1, 192], F32)
        nc.vector.tensor_scalar(out=swRow[:], in0=tj[:], scalar1=0.125,
                                scalar2=None, op0=ALU.mult)
        _floor_inplace(nc, spool, swRow[:], "fr192")
        nc.vector.tensor_tensor(out=ewRow[:], in0=tj[:], in1=lenxB[:],
                                op=ALU.add)
        nc.vector.tensor_scalar(out=ewRow[:], in0=ewRow[:], scalar1=7.0,
                                scalar2=0.125, op0=ALU.add, op1=ALU.mult)
        _floor_inplace(nc, spool, ewRow[:], "fr192")
        nc.vector.tensor_tensor(out=swRow[:], in0=swRow[:], in1=x1cB[:],
                                op=ALU.add)
        nc.vector.tensor_tensor(out=ewRow[:], in0=ewRow[:], in1=x1cB[:],
                                op=ALU.add)
        swB = pool.tile([P, 192], F32)
        ewB = pool.tile([P, 192], F32)
        nc.gpsimd.partition_broadcast(swB[:], swRow[:])
        nc.gpsimd.partition_broadcast(ewB[:], ewRow[:])
        validB = pool.tile([P, 24], F32)
        nc.gpsimd.partition_broadcast(validB[:], validRow[:])

        # ------------- H-stage bins in T-layout (16 partitions = i) -------
        iio = pool.tile([16, 8], I32)
        nc.gpsimd.iota(iio[:], pattern=[[0, 8]], base=0, channel_multiplier=1)
        iiof = pool.tile([16, 8], F32)
        nc.any.tensor_copy(out=iiof[:], in_=iio[:])

        idxF = pool.tile([16, 192], F32)   # gather-unit index, [lvl, b, c]
        maskT = pool.tile([16, 40], F32)   # [mendL0|mendL1|mendL2|mstartL2|-]
        for lvl, (H, W, TG, RU) in enumerate(LVLS):
            lyB = spool.tile([16, 8], F32, tag="lyB")
            nc.gpsimd.partition_broadcast(
                lyB[:], lenRow[:, lvl * 16: lvl * 16 + 8])
            y1B = spool.tile([16, 8], F32, tag="y1B")
            nc.gpsimd.partition_broadcast(y1B[:], crd(lvl, 1))
            ti = spool.tile([16, 8], F32, tag="ti")
            nc.vector.tensor_tensor(out=ti[:], in0=iiof[:], in1=lyB[:],
                                    op=ALU.mult)
            lo = spool.tile([16, 8], F32, tag="lo")
            nc.vector.tensor_scalar(out=lo[:], in0=ti[:], scalar1=0.0625,
                                    scalar2=None, op0=ALU.mult)
            _floor_inplace(nc, spool, lo[:], "fr16")
            hi = spool.tile([16, 8], F32, tag="hi")
            nc.vector.tensor_tensor(out=hi[:], in0=ti[:], in1=lyB[:],
                                    op=ALU.add)
            nc.vector.tensor_scalar(out=hi[:], in0=hi[:], scalar1=15.0,
                                    scalar2=0.0625, op0=ALU.add, op1=ALU.mult)
            _floor_inplace(nc, spool, hi[:], "fr16")
            st = spool.tile([16, 8], F32, tag="st")
            nc.vector.tensor_tensor(out=st[:], in0=y1B[:], in1=lo[:],
                                    op=ALU.add)
            lenT = spool.tile([16, 8], F32, tag="lenT")
            nc.vector.tensor_tensor(out=lenT[:], in0=hi[:], in1=lo[:],
                                    op=ALU.subtract)
            if RU == 1:
                nc.vector.tensor_copy(out=maskT[:, lvl * 8:(lvl + 1) * 8],
                                      in_=lenT[:])
                unit = st
            else:
                half = spool.tile([16, 8], F32, tag="half")
                nc.vector.tensor_scalar(out=half[:], in0=st[:], scalar1=0.5,
                                        scalar2=None, op0=ALU.mult)
                _floor_inplace(nc, spool, half[:], "fr16")
                par = spool.tile([16, 8], F32, tag="par")
                nc.vector.tensor_scalar(out=par[:], in0=half[:], scalar1=-2.0,
                                        scalar2=None, op0=ALU.mult)
                nc.vector.tensor_tensor(out=par[:], in0=par[:], in1=st[:],
                                        op=ALU.add)
                nc.vector.tensor_copy(out=maskT[:, 16:24], in_=par[:])
                nc.vector.tensor_tensor(out=maskT[:, 24:32], in0=par[:],
                                        in1=lenT[:], op=ALU.add)
                unit = half
            # idxF[i, lvl*64 + b*8 + c] = unit(i, b) + (H/RU)*(8*b + c)
            upc = H // RU
            bio = spool.tile([16, 64], I32, tag="bio")
            nc.gpsimd.iota(bio[:], pattern=[[upc * 8, 8], [upc, 8]], base=0,
                           channel_multiplier=0)
            biof = spool.tile([16, 64], F32, tag="biof")
            nc.any.tensor_copy(out=biof[:], in_=bio[:])
            nc.vector.tensor_tensor(
                out=idxF[:, lvl * 64:(lvl + 1) * 64].rearrange(
                    "p (b c) -> p b c", c=8),
                in0=biof[:].rearrange("p (b c) -> p b c", c=8),
                in1=unit[:].unsqueeze(2).to_broadcast([16, 8, 8]),
                op=ALU.add)

        idx16r = pool.tile([16, 192], I16)
        nc.any.tensor_copy(out=idx16r[:], in_=idxF[:])
        idx16 = pool.tile([P, 192], I16)
        maskF = pool.tile([P, 40], F32)
        for g in range(8):
            nc.sync.dma_start(idx16[16 * g:16 * (g + 1), :], idx16r[:])
            nc.sync.dma_start(maskF[16 * g:16 * (g + 1), :], maskT[:])

        import os as _os
        _stage = _os.environ.get("KSTAGE", "full")
        if _stage == "box":
            dbg = pool.tile([8, 238], F32)
            nc.vector.memset(dbg[:], 0.0)
            nc.vector.tensor_copy(out=dbg[:4, :8], in_=boxT[:])
            nc.sync.dma_start(out_d[:], dbg[:])
            return nc

        # ------------- pooling -------------------------------------------
        # Static per-level iotas: t index (for the row mask) and per-j w index
        falls = {}
        for lvl, (H, W, TG, RU) in enumerate(LVLS):
            upc = H // RU
            n_units = B * CPC * upc + (PADROWS - TG) // RU
            in_gather = bass.AP(xs_dram[lvl].ap().tensor, 0,
                                [[W * RU, n_units], [1, TG * W]])
            iti = spool.tile([P, TG], I32, tag="iti")
            nc.gpsimd.iota(iti[:], pattern=[[1, TG]], base=0,
                           channel_multiplier=0)
            itf = pool.tile([P, TG], F32, name=f"itf{lvl}")
            nc.any.tensor_copy(out=itf[:], in_=iti[:])
            iwi = spool.tile([P, POOL_W * W], I32, tag="iwi")
            nc.gpsimd.iota(iwi[:], pattern=[[0, POOL_W], [1, W]], base=0,
                           channel_multiplier=0)
            iwf = pool.tile([P, POOL_W * W], F32, name=f"iwf{lvl}")
            nc.any.tensor_copy(out=iwf[:], in_=iwi[:])
            fall = pool.tile([P, B, POOL_W], F32, name=f"fall{lvl}")
            falls[lvl] = fall
            for b in range(B):
                g = gpool.tile([P, TG * W], F32, tag="g")
                if _stage == "poolng":
                    nc.vector.memset(g[:], 0.0)
                else:
                    nc.gpsimd.dma_gather(
                        out_ap=g[:].unsqueeze(1),
                        in_ap=in_gather,
                        idxs_ap=idx16[:, lvl * 64 + b * 8: lvl * 64 + b * 8 + 8],
                        num_idxs=P, num_idxs_reg=P,
                        elem_size=TG * W, elem_step=W * RU)
                # madd[p, t] = 0 if row t is inside this (b, i=p%16) bin
                madd = spool.tile([P, TG], F32, tag="madd")
                if RU == 1:
                    nc.vector.tensor_scalar(
                        out=madd[:], in0=itf[:],
                        scalar1=maskF[:, lvl * 8 + b: lvl * 8 + b + 1],
                        scalar2=None, op0=ALU.is_lt)
                    nc.vector.tensor_scalar(
                        out=madd[:], in0=madd[:], scalar1=1.0, scalar2=1e30,
                        op0=ALU.subtract, op1=ALU.mult)
                else:
                    m2 = spool.tile([P, TG], F32, tag="m2x3")
                    nc.vector.tensor_scalar(
                        out=madd[:], in0=itf[:],
                        scalar1=maskF[:, 16 + b: 17 + b],
                        scalar2=None, op0=ALU.is_ge)
                    nc.vector.tensor_scalar(
                        out=m2[:], in0=itf[:],
                        scalar1=maskF[:, 24 + b: 25 + b],
                        scalar2=None, op0=ALU.is_lt)
                    nc.vector.tensor_tensor(out=madd[:], in0=madd[:],
                                            in1=m2[:], op=ALU.add)
                    nc.vector.tensor_scalar(
                        out=madd[:], in0=madd[:], scalar1=2.0, scalar2=1e30,
                        op0=ALU.subtract, op1=ALU.mult)
                # H-stage: running max over masked rows
                s1h = gpool.tile([P, W], F32, tag="s1h")
                nc.vector.tensor_scalar(out=s1h[:], in0=g[:, 0:W],
                                        scalar1=madd[:, 0:1], scalar2=None,
                                        op0=ALU.add)
                for t in range(1, TG):
                    tmp = gpool.tile([P, W], F32, tag="tmp")
                    nc.vector.tensor_scalar(out=tmp[:],
                                            in0=g[:, t * W:(t + 1) * W],
                                            scalar1=madd[:, t:t + 1],
                                            scalar2=None, op0=ALU.add)
                    nc.vector.tensor_tensor(out=s1h[:], in0=s1h[:],
                                            in1=tmp[:], op=ALU.max)
                # W-stage: all 8 column bins at once on the (8j, W) grid
                ge = spool.tile([P, POOL_W, W], F32, tag="ge")
                nc.vector.tensor_tensor(
                    out=ge[:],
                    in0=iwf[:].rearrange("p (j w) -> p j w", w=W),
                    in1=swB[:, lvl * 64 + b * 8: lvl * 64 + b * 8 + 8]
                    .unsqueeze(2).to_broadcast([P, POOL_W, W]),
                    op=ALU.is_ge)
                lt = spool.tile([P, POOL_W, W], F32, tag="lt")
                nc.vector.tensor_tensor(
                    out=lt[:],
                    in0=iwf[:].rearrange("p (j w) -> p j w", w=W),
                    in1=ewB[:, lvl * 64 + b * 8: lvl * 64 + b * 8 + 8]
                    .unsqueeze(2).to_broadcast([P, POOL_W, W]),
                    op=ALU.is_lt)
                nc.vector.tensor_tensor(out=ge[:], in0=ge[:], in1=lt[:],
                                        op=ALU.add)
                nc.vector.tensor_scalar(
                    out=ge[:].rearrange("p j w -> p (j w)"),
                    in0=ge[:].rearrange("p j w -> p (j w)"),
                    scalar1=2.0, scalar2=1e30, op0=ALU.subtract, op1=ALU.mult)
                nc.vector.tensor_tensor(
                    out=ge[:], in0=ge[:],
                    in1=s1h[:].unsqueeze(1).to_broadcast([P, POOL_W, W]),
                    op=ALU.add)
                nc.vector.tensor_reduce(out=fall[:, b, :], in_=ge[:],
                                        axis=AXX.X, op=ALU.max)
        for lvl in range(3):
            nc.vector.tensor_tensor(
                out=falls[lvl][:], in0=falls[lvl][:],
                in1=validB[:, lvl * 8: lvl * 8 + 8]
                .unsqueeze(2).to_broadcast([P, B, POOL_W]), op=ALU.mult)

        if _stage in ("pool", "poolng"):
            dbg = pool.tile([8, 238], F32)
            nc.vector.memset(dbg[:], 0.0)
            nc.vector.tensor_copy(out=dbg[:8, :8],
                                  in_=falls[0][:8, 0, :].unsqueeze(1))
            nc.sync.dma_start(out_d[:], dbg[:])
            return nc

        # ------------- head matmuls + AllReduce ---------------------------
        hps = [hpsp.tile([P, B], F32, tag=f"hps{h}", name=f"hps{h}")
               for h in range(7)]
        for chunk in range(24):
            Fj = falls[chunk // 8][:, :, chunk % 8]
            for h in range(7):
                nc.tensor.matmul(
                    out=hps[h][:], lhsT=whp[:, chunk, h * P:(h + 1) * P],
                    rhs=Fj, start=(chunk == 0), stop=(chunk == 23))
        hcat = pool.tile([P, 7 * B], F32)
        for h in range(7):
            nc.any.tensor_copy(out=hcat[:, h * B:(h + 1) * B], in_=hps[h][:])
        cc_in2 = dram.tile([P, 7 * B], F32)
        cc_out2 = dram.tile([P, 7 * B], F32)
        nc.sync.dma_start(cc_in2[:], hcat[:])
        nc.gpsimd.collective_compute(
            "AllReduce", ALU.add, replica_groups=[list(range(NCORES))],
            ins=[cc_in2.opt()], outs=[cc_out2.opt()])
        hsum = pool.tile([P, 7 * B], F32)
        nc.sync.dma_start(hsum[:], cc_out2[:])

        # ------------- relu(h + b1) and the 7 output heads ----------------
        b1t = pool.tile([P, 7], F32)
        nc.sync.dma_start(b1t[:], b1t_d[:])
        w2c = pool.tile([P, 238], F32)
        nc.sync.dma_start(w2c[:], w2c_d[:])
        b2r = pool.tile([8, 238], F32)
        nc.sync.dma_start(b2r[:], b2r_d[:])
        hrelu = pool.tile([P, 7 * B], F32)
        for h in range(7):
            nc.scalar.activation(out=hrelu[:, h * B:(h + 1) * B],
                                 in_=hsum[:, h * B:(h + 1) * B],
                                 func=ACTF.Relu, bias=b1t[:, h:h + 1],
                                 scale=1.0)
        outs_sb = pool.tile([8, 238], F32)
        offs = [0, 38, 63, 98, 133, 168, 203, 238]
        for h in range(7):
            n_h = offs[h + 1] - offs[h]
            pf = sps.tile([8, 512], F32, tag="sps", name=f"pf{h}")
            nc.tensor.matmul(out=pf[:, :n_h],
                             lhsT=hrelu[:, h * B:(h + 1) * B],
                             rhs=w2c[:, offs[h]:offs[h + 1]],
                             start=True, stop=True)
            nc.vector.tensor_add(out=outs_sb[:, offs[h]:offs[h + 1]],
                                 in0=pf[:, :n_h],
                                 in1=b2r[:, offs[h]:offs[h + 1]])
        nc.sync.dma_start(out_d[:], outs_sb[:])
    return nc


# ---------------------------------------------------------------------------
# host side
# ---------------------------------------------------------------------------
_CACHE = {}


def _get_nc():
    if "nc" not in _CACHE:
        nc = bacc.Bacc("TRN2", target_bir_lowering=False, debug=False,
                       num_devices=NCORES)
        _build(nc)
        nc.compile()
        _CACHE["nc"] = nc
    return _CACHE["nc"]


def _fold_weights(inp):
    w2, b2, w3, b3 = (inp["wr2_w2"], inp["wr2_b2"], inp["wr2_w3"],
                      inp["wr2_b3"])
    b1 = inp["wr2_b1"]
    postfix = np.array(
        [[1.0, 0.0, 1.0, 0.0], [0.0, 1.0, 0.0, 1.0],
         [-0.5, 0.0, 0.5, 0.0], [0.0, -0.5, 0.0, 0.5]], np.float32)
    Wf = np.ascontiguousarray((w2 @ w3 @ postfix).astype(np.float32))
    bf = ((((b1 @ w2 + b2) @ w3) + b3) @ postfix).astype(np.float32)
    return Wf, bf


def _shard_inputs(inp):
    inp = {k: np.ascontiguousarray(np.asarray(v)) for k, v in inp.items()}
    xs = {0: inp["x1"][..., 0], 1: inp["x2"][..., 0], 2: inp["x3"][..., 0]}
    Wf, bf = _fold_weights(inp)
    bf_rep = np.ascontiguousarray(np.repeat(bf[:, None], 8, axis=1))
    scl = np.zeros((1, 96), np.float32)
    for lvl, (H, W, TG, RU) in enumerate(LVLS):
        for ci, s in enumerate([W, H, W, H]):
            scl[0, lvl * 32 + ci * 8: lvl * 32 + ci * 8 + 8] = s
    b1t = np.ascontiguousarray(inp["head_b1"].T)         # (128, 7)
    w2c = np.ascontiguousarray(np.concatenate(
        [inp["w2_prov"], inp["w2_alpha"]]
        + [inp["w2_ad"][t] for t in range(5)], axis=1))  # (128, 238)
    b2c = np.concatenate(
        [inp["b2_prov"], inp["b2_alpha"]]
        + [inp["b2_ad"][t] for t in range(5)])           # (238,)
    b2r = np.ascontiguousarray(np.repeat(b2c[None, :], 8, axis=0))

    hw1 = inp["head_w1"].reshape(7, 3, 64, POOL_H, POOL_W, P)
    w1 = inp["wr2_w1"]

    in_maps = []
    for k in range(NCORES):
        m = {"wf": Wf, "bf": bf_rep, "scl": scl, "b1t": b1t, "w2c": w2c,
             "b2r": b2r}
        for li, (H, W, TG, RU) in enumerate(LVLS):
            flat = xs[li][:, CPC * k:CPC * (k + 1)].reshape(-1, W)
            m[f"xs{li}"] = np.ascontiguousarray(
                np.concatenate([flat, np.zeros((PADROWS, W), np.float32)], 0))
        x3 = xs[2][:, CPC * k:CPC * (k + 1)].reshape(B, 8, 8, P)
        m["x3t"] = np.ascontiguousarray(
            x3.transpose(3, 1, 2, 0).reshape(P, 64 * B))
        w1s = w1[CPC * k * 1024:CPC * (k + 1) * 1024].reshape(8, 8, P, 100)
        m["w1p"] = np.ascontiguousarray(
            w1s.transpose(2, 0, 1, 3).reshape(P, 64 * 100))
        whs = hw1[:, :, CPC * k:CPC * (k + 1)]     # (7, 3, 8c, 16i, 8j, 128)
        m["whp"] = np.ascontiguousarray(
            whs.transpose(2, 3, 1, 4, 0, 5).reshape(P, 24 * D_HEADS))
        in_maps.append(m)
    return in_maps


def kernel(**inputs):
    nc = _get_nc()
    in_maps = _shard_inputs(inputs)
    res = run_bass_kernel_spmd(nc, in_maps, core_ids=list(range(NCORES)))
    _CACHE["last_res"] = res
    out = np.asarray(res.results[0]["out_all"], np.float32)
    o1, o2 = out[:, 0:38], out[:, 38:63]
    oad = [out[:, 63 + 35 * t: 63 + 35 * (t + 1)] for t in range(5)]
    return (o1, o2, *oad)


# revision 11
# speedup vs baseline: 1.0048x; 1.0048x over previous
"""Trainium2 Bass kernel for nn_CombinedModel_15977278341388 (nms_detection).

Sharding (8 NeuronCores, memory-bound):
  * wr2 layer-1 matmul (65536x100, 26MB): contraction sharded by channel;
    (100,B) partials AllReduce'd on-device (3.2KB).
  * wr2 layers 2/3 + postfix folded on host into one (100,4) matmul (weight
    preprocessing only -- no input data touched on host).
  * ROI adaptive max pool: channel-sharded (each core pools its 8 channels of
    all samples / all 3 pyramid levels).  Row windows are fetched straight
    from HBM with dma_gather (per-(c,i) row blocks), trailing garbage rows
    masked via tensor_mask_reduce's select-to -FLT_MAX, then a static
    tensor_reduce max; column bins via tensor_mask_reduce windows whose
    per-(b,j) bounds are computed on-device from the predicted boxes.
  * head_w1 (7x24576x128, 88MB): contraction-dim sharded to exactly the
    pooled channels each core owns -> per-core (128hcol, 7*8) partials,
    one 28KB AllReduce, then ReLU + 7 small output matmuls (replicated).

kernel(**inputs) takes FULL inputs, shards internally, returns the FULL
7-tuple (o1, o2, ad0..ad4).
"""
import numpy as np
from contextlib import ExitStack

import concourse.bass as bass
import concourse.mybir as mybir
import concourse.tile as tile
from concourse import bacc
from concourse.bass_utils import run_bass_kernel_spmd

F32 = mybir.dt.float32
I16 = mybir.dt.int16
I32 = mybir.dt.int32
ALU = mybir.AluOpType
ACTF = mybir.ActivationFunctionType
AXX = mybir.AxisListType

P = 128
B = 8
NCORES = 8
CPC = 8            # channels per core
POOL_H, POOL_W = 16, 8
NEGF = float(np.finfo(np.float32).min)

# (H, W, gather_rows, row_unit).  x3 uses 2-row units so the dma_gather
# stride is 256B (the HW minimum); a 4-row window + parity mask covers any
# 3-row bin window.
LVLS = [(128, 128, 9, 1), (64, 64, 5, 1), (32, 32, 4, 2)]
PADROWS = 16
D_HEADS = 7 * 128  # 896


def _floor_inplace(nc, spool, ap, tag):
    """ap <- floor(ap) for non-negative f32 values (|x| < 2^31).

    f32->i32->f32 round trip gives either trunc(x) or rne(x) depending on
    the cast mode; both land in {floor(x), floor(x)+1}, so subtracting the
    (rounded > x) flag yields an exact floor under either semantics.
    """
    ti = spool.tile(list(ap.shape), I32, tag=tag + "i")
    nc.vector.tensor_copy(out=ti[:], in_=ap)
    tf = spool.tile(list(ap.shape), F32, tag=tag + "f")
    nc.vector.tensor_copy(out=tf[:], in_=ti[:])
    gt = spool.tile(list(ap.shape), F32, tag=tag + "g")
    nc.vector.tensor_tensor(out=gt[:], in0=tf[:], in1=ap, op=ALU.is_gt)
    nc.vector.tensor_tensor(out=ap, in0=tf[:], in1=gt[:], op=ALU.subtract)


def _build(nc):
    # ---------------- DRAM I/O ----------------
    xs_dram = [
        nc.dram_tensor(f"xs{li}", [B * CPC * H + PADROWS, W], F32,
                       kind="ExternalInput")
        for li, (H, W, TG, RU) in enumerate(LVLS)
    ]
    x3t_d = nc.dram_tensor("x3t", [P, 64 * B], F32, kind="ExternalInput")
    w1p_d = nc.dram_tensor("w1p", [P, 64 * 100], F32, kind="ExternalInput")
    whp_d = nc.dram_tensor("whp", [P, 24 * D_HEADS], F32, kind="ExternalInput")
    wf_d = nc.dram_tensor("wf", [100, 4], F32, kind="ExternalInput")
    bf_d = nc.dram_tensor("bf", [4, 8], F32, kind="ExternalInput")
    scl_d = nc.dram_tensor("scl", [1, 96], F32, kind="ExternalInput")
    b1t_d = nc.dram_tensor("b1t", [P, 7], F32, kind="ExternalInput")
    w2c_d = nc.dram_tensor("w2c", [P, 238], F32, kind="ExternalInput")
    b2r_d = nc.dram_tensor("b2r", [8, 238], F32, kind="ExternalInput")
    out_d = nc.dram_tensor("out_all", [8, 238], F32, kind="ExternalOutput")

    with tile.TileContext(nc) as tc, ExitStack() as ctx:
        pool = ctx.enter_context(tc.tile_pool(name="main", bufs=1))
        gpool = ctx.enter_context(tc.tile_pool(name="gath", bufs=3))
        spool = ctx.enter_context(tc.tile_pool(name="scr", bufs=2))
        hpsp = ctx.enter_context(tc.tile_pool(name="hps", bufs=1, space="PSUM"))
        sps = ctx.enter_context(tc.tile_pool(name="sps", bufs=1, space="PSUM"))
        dram = ctx.enter_context(tc.tile_pool(name="dram", bufs=1, space="DRAM"))

        # ------------- head weights: stream in early, used late -----------
        whp = pool.tile([P, 24, D_HEADS], F32)
        for blk in range(8):
            nc.sync.dma_start(
                whp[:, 3 * blk:3 * (blk + 1)],
                whp_d[:, 3 * blk * D_HEADS:3 * (blk + 1) * D_HEADS]
                .rearrange("p (c n) -> p c n", n=D_HEADS))

        # ------------- stage A: h1 partial = x3_shard @ W1_shard ----------
        x3t = pool.tile([P, 64, B], F32)
        nc.sync.dma_start(x3t[:], x3t_d[:].rearrange("p (c b) -> p c b", b=B))
        w1p = pool.tile([P, 64, 100], F32)
        nc.sync.dma_start(w1p[:], w1p_d[:].rearrange("p (c o) -> p c o", o=100))
        h1ps = sps.tile([100, B], F32, tag="sps", name="h1ps")
        for ch in range(64):
            nc.tensor.matmul(out=h1ps[:], lhsT=w1p[:, ch], rhs=x3t[:, ch],
                             start=(ch == 0), stop=(ch == 63))
        h1sb = pool.tile([100, B], F32)
        nc.any.tensor_copy(out=h1sb[:], in_=h1ps[:])

        cc_in1 = dram.tile([100, B], F32)
        cc_out1 = dram.tile([100, B], F32)
        nc.sync.dma_start(cc_in1[:], h1sb[:])
        nc.gpsimd.collective_compute(
            "AllReduce", ALU.add, replica_groups=[list(range(NCORES))],
            ins=[cc_in1.opt()], outs=[cc_out1.opt()])
        h1t = pool.tile([100, B], F32)
        nc.sync.dma_start(h1t[:], cc_out1[:])

        # ------------- boxes: boxT = clip(h1 @ Wf + bf, 0, 1) -------------
        wf = pool.tile([100, 4], F32)
        nc.sync.dma_start(wf[:], wf_d[:])
        bfr = pool.tile([4, 8], F32)
        nc.sync.dma_start(bfr[:], bf_d[:])
        boxps = sps.tile([4, B], F32, tag="sps", name="boxps")
        nc.tensor.matmul(out=boxps[:], lhsT=wf[:], rhs=h1t[:],
                         start=True, stop=True)
        boxT = pool.tile([4, B], F32)
        nc.vector.tensor_add(out=boxT[:], in0=boxps[:], in1=bfr[:])
        nc.vector.tensor_scalar(out=boxT[:], in0=boxT[:], scalar1=0.0,
                                scalar2=1.0, op0=ALU.max, op1=ALU.min)

        # cRow (1,32) = [x1(8) | y1(8) | x2(8) | y2(8)] on partition 0
        cRow = pool.tile([1, 32], F32)
        nc.sync.dma_start(cRow[:].rearrange("p (c b) -> p c b", b=B), boxT[:])

        # ------------- integer coords for the 3 levels --------------------
        scl = pool.tile([1, 96], F32)      # [lvl, coord, b] = W/H/W/H
        nc.sync.dma_start(scl[:], scl_d[:])
        cAll = pool.tile([1, 96], F32)
        nc.vector.tensor_tensor(
            out=cAll[:].rearrange("p (l c) -> p l c", l=3),
            in0=cRow[:].unsqueeze(1).to_broadcast([1, 3, 32]),
            in1=scl[:].rearrange("p (l c) -> p l c", l=3), op=ALU.mult)
        _floor_inplace(nc, spool, cAll[:], "fr96")

        def crd(lvl, c):  # (1, 8) coordinate row of a level
            return cAll[:, lvl * 32 + c * 8: lvl * 32 + c * 8 + 8]

        # lenRow (1,48): [lvl][leny(8) | lenx(8)];  validRow (1,24): [lvl, b]
        lenRow = pool.tile([1, 48], F32)
        validRow = pool.tile([1, 24], F32)
        for lvl, (H, W, TG, RU) in enumerate(LVLS):
            ly = lenRow[:, lvl * 16: lvl * 16 + 8]
            lx = lenRow[:, lvl * 16 + 8: lvl * 16 + 16]
            nc.vector.tensor_tensor(out=ly, in0=crd(lvl, 3), in1=crd(lvl, 1),
                                    op=ALU.subtract)
            nc.vector.tensor_scalar(out=ly, in0=ly, scalar1=1.0, scalar2=1.0,
                                    op0=ALU.add, op1=ALU.max)
            nc.vector.tensor_tensor(out=lx, in0=crd(lvl, 2), in1=crd(lvl, 0),
                                    op=ALU.subtract)
            nc.vector.tensor_scalar(out=lx, in0=lx, scalar1=1.0, scalar2=1.0,
                                    op0=ALU.add, op1=ALU.max)
            vr = validRow[:, lvl * 8: lvl * 8 + 8]
            nc.vector.tensor_scalar(out=vr, in0=crd(lvl, 2), scalar1=float(W),
                                    scalar2=None, op0=ALU.is_lt)
            for in0, in1, op in ((crd(lvl, 3), float(H), ALU.is_lt),
                                 (crd(lvl, 2), crd(lvl, 0), ALU.is_gt),
                                 (crd(lvl, 3), crd(lvl, 1), ALU.is_gt)):
                vv = spool.tile([1, 8], F32, tag="vv")
                if isinstance(in1, float):
                    nc.vector.tensor_scalar(out=vv[:], in0=in0, scalar1=in1,
                                            scalar2=None, op0=op)
                else:
                    nc.vector.tensor_tensor(out=vv[:], in0=in0, in1=in1, op=op)
                nc.vector.tensor_tensor(out=vr, in0=vr, in1=vv[:], op=ALU.mult)

        # ------------- W-stage bounds rows (1, 192) -----------------------
        # col bin j of (lvl, b): [x1c + floor(j*lenx/8),
        #                         x1c + floor((j*lenx + lenx + 7)/8))
        jio = pool.tile([1, 192], I32)
        nc.gpsimd.iota(jio[:], pattern=[[0, 24], [1, 8]], base=0,
                       channel_multiplier=0)
        jiof = pool.tile([1, 192], F32)
        nc.any.tensor_copy(out=jiof[:], in_=jio[:])
        lenxB = pool.tile([1, 192], F32)
        x1cB = pool.tile([1, 192], F32)
        for lvl in range(3):
            nc.any.tensor_copy(
                out=lenxB[:, lvl * 64:(lvl + 1) * 64].rearrange(
                    "p (b j) -> p b j", j=8),
                in_=lenRow[:, lvl * 16 + 8: lvl * 16 + 16]
                .unsqueeze(2).to_broadcast([1, 8, 8]))
            nc.any.tensor_copy(
                out=x1cB[:, lvl * 64:(lvl + 1) * 64].rearrange(
                    "p (b j) -> p b j", j=8),
                in_=crd(lvl, 0).unsqueeze(2).to_broadcast([1, 8, 8]))
        tj = pool.tile([1, 192], F32)
        nc.vector.tensor_tensor(out=tj[:], in0=jiof[:], in1=lenxB[:],
                                op=ALU.mult)
        swRow = pool.tile([1, 192], F32)
        ewRow = pool.tile([1, 192], F32)
        nc.vector.tensor_scalar(out=swRow[:], in0=tj[:], scalar1=0.125,
                                scalar2=None, op0=ALU.mult)
        _floor_inplace(nc, spool, swRow[:], "fr192")
        nc.vector.tensor_tensor(out=ewRow[:], in0=tj[:], in1=lenxB[:],
                                op=ALU.add)
        nc.vector.tensor_scalar(out=ewRow[:], in0=ewRow[:], scalar1=7.0,
                                scalar2=0.125, op0=ALU.add, op1=ALU.mult)
        _floor_inplace(nc, spool, ewRow[:], "fr192")
        nc.vector.tensor_tensor(out=swRow[:], in0=swRow[:], in1=x1cB[:],
                                op=ALU.add)
        nc.vector.tensor_tensor(out=ewRow[:], in0=ewRow[:], in1=x1cB[:],
                                op=ALU.add)
        swB = pool.tile([P, 192], F32)
        ewB = pool.tile([P, 192], F32)
        nc.gpsimd.partition_broadcast(swB[:], swRow[:])
        nc.gpsimd.partition_broadcast(ewB[:], ewRow[:])
        validB = pool.tile([P, 24], F32)
        nc.gpsimd.partition_broadcast(validB[:], validRow[:])

        # ------------- H-stage bins in T-layout (16 partitions = i) -------
        iio = pool.tile([16, 8], I32)
        nc.gpsimd.iota(iio[:], pattern=[[0, 8]], base=0, channel_multiplier=1)
        iiof = pool.tile([16, 8], F32)
        nc.any.tensor_copy(out=iiof[:], in_=iio[:])

        idxF = pool.tile([16, 192], F32)   # gather-unit index, [lvl, b, c]
        maskT = pool.tile([16, 40], F32)   # [mendL0|mendL1|mendL2|mstartL2|-]
        for lvl, (H, W, TG, RU) in enumerate(LVLS):
            lyB = spool.tile([16, 8], F32, tag="lyB")
            nc.gpsimd.partition_broadcast(
                lyB[:], lenRow[:, lvl * 16: lvl * 16 + 8])
            y1B = spool.tile([16, 8], F32, tag="y1B")
            nc.gpsimd.partition_broadcast(y1B[:], crd(lvl, 1))
            ti = spool.tile([16, 8], F32, tag="ti")
            nc.vector.tensor_tensor(out=ti[:], in0=iiof[:], in1=lyB[:],
                                    op=ALU.mult)
            lo = spool.tile([16, 8], F32, tag="lo")
            nc.vector.tensor_scalar(out=lo[:], in0=ti[:], scalar1=0.0625,
                                    scalar2=None, op0=ALU.mult)
            _floor_inplace(nc, spool, lo[:], "fr16")
            hi = spool.tile([16, 8], F32, tag="hi")
            nc.vector.tensor_tensor(out=hi[:], in0=ti[:], in1=lyB[:],
                                    op=ALU.add)
            nc.vector.tensor_scalar(out=hi[:], in0=hi[:], scalar1=15.0,
                                    scalar2=0.0625, op0=ALU.add, op1=ALU.mult)
            _floor_inplace(nc, spool, hi[:], "fr16")
            st = spool.tile([16, 8], F32, tag="st")
            nc.vector.tensor_tensor(out=st[:], in0=y1B[:], in1=lo[:],
                                    op=ALU.add)
            lenT = spool.tile([16, 8], F32, tag="lenT")
            nc.vector.tensor_tensor(out=lenT[:], in0=hi[:], in1=lo[:],
                                    op=ALU.subtract)
            if RU == 1:
                nc.vector.tensor_copy(out=maskT[:, lvl * 8:(lvl + 1) * 8],
                                      in_=lenT[:])
                unit = st
            else:
                half = spool.tile([16, 8], F32, tag="half")
                nc.vector.tensor_scalar(out=half[:], in0=st[:], scalar1=0.5,
                                        scalar2=None, op0=ALU.mult)
                _floor_inplace(nc, spool, half[:], "fr16")
                par = spool.tile([16, 8], F32, tag="par")
                nc.vector.tensor_scalar(out=par[:], in0=half[:], scalar1=-2.0,
                                        scalar2=None, op0=ALU.mult)
                nc.vector.tensor_tensor(out=par[:], in0=par[:], in1=st[:],
                                        op=ALU.add)
                nc.vector.tensor_copy(out=maskT[:, 16:24], in_=par[:])
                nc.vector.tensor_tensor(out=maskT[:, 24:32], in0=par[:],
                                        in1=lenT[:], op=ALU.add)
                unit = half
            # idxF[i, lvl*64 + b*8 + c] = unit(i, b) + (H/RU)*(8*b + c)
            upc = H // RU
            bio = spool.tile([16, 64], I32, tag="bio")
            nc.gpsimd.iota(bio[:], pattern=[[upc * 8, 8], [upc, 8]], base=0,
                           channel_multiplier=0)
            biof = spool.tile([16, 64], F32, tag="biof")
            nc.any.tensor_copy(out=biof[:], in_=bio[:])
            nc.vector.tensor_tensor(
                out=idxF[:, lvl * 64:(lvl + 1) * 64].rearrange(
                    "p (b c) -> p b c", c=8),
                in0=biof[:].rearrange("p (b c) -> p b c", c=8),
                in1=unit[:].unsqueeze(2).to_broadcast([16, 8, 8]),
                op=ALU.add)

        idx16r = pool.tile([16, 192], I16)
        nc.any.tensor_copy(out=idx16r[:], in_=idxF[:])
        idx16 = pool.tile([P, 192], I16)
        maskF = pool.tile([P, 40], F32)
        for g in range(8):
            nc.sync.dma_start(idx16[16 * g:16 * (g + 1), :], idx16r[:])
            nc.sync.dma_start(maskF[16 * g:16 * (g + 1), :], maskT[:])

        import os as _os
        _stage = _os.environ.get("KSTAGE", "full")
        if _stage == "box":
            dbg = pool.tile([8, 238], F32)
            nc.vector.memset(dbg[:], 0.0)
            nc.vector.tensor_copy(out=dbg[:4, :8], in_=boxT[:])
            nc.sync.dma_start(out_d[:], dbg[:])
            return nc

        # ------------- pooling -------------------------------------------
        # Static per-level iotas: t index (for the row mask) and per-j w index
        falls = {}
        for lvl, (H, W, TG, RU) in enumerate(LVLS):
            upc = H // RU
            n_units = B * CPC * upc + (PADROWS - TG) // RU
            in_gather = bass.AP(xs_dram[lvl].ap().tensor, 0,
                                [[W * RU, n_units], [1, TG * W]])
            iti = spool.tile([P, TG], I32, tag="iti")
            nc.gpsimd.iota(iti[:], pattern=[[1, TG]], base=0,
                           channel_multiplier=0)
            itf = pool.tile([P, TG], F32, name=f"itf{lvl}")
            nc.any.tensor_copy(out=itf[:], in_=iti[:])
            iwi = spool.tile([P, POOL_W * W], I32, tag="iwi")
            nc.gpsimd.iota(iwi[:], pattern=[[0, POOL_W], [1, W]], base=0,
                           channel_multiplier=0)
            iwf = pool.tile([P, POOL_W * W], F32, name=f"iwf{lvl}")
            nc.any.tensor_copy(out=iwf[:], in_=iwi[:])
            fall = pool.tile([P, B, POOL_W], F32, name=f"fall{lvl}")
            falls[lvl] = fall
            for b in range(B):
                g = gpool.tile([P, TG * W], F32, tag="g")
                if _stage == "poolng":
                    nc.vector.memset(g[:], 0.0)
                else:
                    nc.gpsimd.dma_gather(
                        out_ap=g[:].unsqueeze(1),
                        in_ap=in_gather,
                        idxs_ap=idx16[:, lvl * 64 + b * 8: lvl * 64 + b * 8 + 8],
                        num_idxs=P, num_idxs_reg=P,
                        elem_size=TG * W, elem_step=W * RU)
                # madd[p, t] = 0 if row t is inside this (b, i=p%16) bin
                madd = spool.tile([P, TG], F32, tag="madd")
                if RU == 1:
                    nc.vector.tensor_scalar(
                        out=madd[:], in0=itf[:],
                        scalar1=maskF[:, lvl * 8 + b: lvl * 8 + b + 1],
                        scalar2=None, op0=ALU.is_lt)
                    nc.vector.tensor_scalar(
                        out=madd[:], in0=madd[:], scalar1=1.0, scalar2=1e30,
                        op0=ALU.subtract, op1=ALU.mult)
                else:
                    m2 = spool.tile([P, TG], F32, tag="m2x3")
                    nc.vector.tensor_scalar(
                        out=madd[:], in0=itf[:],
                        scalar1=maskF[:, 16 + b: 17 + b],
                        scalar2=None, op0=ALU.is_ge)
                    nc.vector.tensor_scalar(
                        out=m2[:], in0=itf[:],
                        scalar1=maskF[:, 24 + b: 25 + b],
                        scalar2=None, op0=ALU.is_lt)
                    nc.vector.tensor_tensor(out=madd[:], in0=madd[:],
                                            in1=m2[:], op=ALU.add)
                    nc.vector.tensor_scalar(
                        out=madd[:], in0=madd[:], scalar1=2.0, scalar2=1e30,
                        op0=ALU.subtract, op1=ALU.mult)
                # H-stage: one masked add over (t, w), then a max chain
                sel = gpool.tile([P, TG * W], F32, tag="sel")
                nc.vector.tensor_tensor(
                    out=sel[:].rearrange("p (t w) -> p t w", w=W),
                    in0=g[:].rearrange("p (t w) -> p t w", w=W),
                    in1=madd[:].unsqueeze(2).to_broadcast([P, TG, W]),
                    op=ALU.add)
                s1h = gpool.tile([P, W], F32, tag="s1h")
                nc.vector.tensor_tensor(out=s1h[:], in0=sel[:, 0:W],
                                        in1=sel[:, W:2 * W], op=ALU.max)
                for t in range(2, TG):
                    nc.vector.tensor_tensor(out=s1h[:], in0=s1h[:],
                                            in1=sel[:, t * W:(t + 1) * W],
                                            op=ALU.max)
                # W-stage: all 8 column bins at once on the (8j, W) grid
                ge = spool.tile([P, POOL_W, W], F32, tag="ge")
                nc.vector.tensor_tensor(
                    out=ge[:],
                    in0=iwf[:].rearrange("p (j w) -> p j w", w=W),
                    in1=swB[:, lvl * 64 + b * 8: lvl * 64 + b * 8 + 8]
                    .unsqueeze(2).to_broadcast([P, POOL_W, W]),
                    op=ALU.is_ge)
                lt = spool.tile([P, POOL_W, W], F32, tag="lt")
                nc.vector.tensor_tensor(
                    out=lt[:],
                    in0=iwf[:].rearrange("p (j w) -> p j w", w=W),
                    in1=ewB[:, lvl * 64 + b * 8: lvl * 64 + b * 8 + 8]
                    .unsqueeze(2).to_broadcast([P, POOL_W, W]),
                    op=ALU.is_lt)
                nc.vector.tensor_tensor(out=ge[:], in0=ge[:], in1=lt[:],
                                        op=ALU.add)
                nc.vector.tensor_scalar(
                    out=ge[:].rearrange("p j w -> p (j w)"),
                    in0=ge[:].rearrange("p j w -> p (j w)"),
                    scalar1=2.0, scalar2=1e30, op0=ALU.subtract, op1=ALU.mult)
                nc.vector.tensor_tensor(
                    out=ge[:], in0=ge[:],
                    in1=s1h[:].unsqueeze(1).to_broadcast([P, POOL_W, W]),
                    op=ALU.add)
                nc.vector.tensor_reduce(out=fall[:, b, :], in_=ge[:],
                                        axis=AXX.X, op=ALU.max)
        for lvl in range(3):
            nc.vector.tensor_tensor(
                out=falls[lvl][:], in0=falls[lvl][:],
                in1=validB[:, lvl * 8: lvl * 8 + 8]
                .unsqueeze(2).to_broadcast([P, B, POOL_W]), op=ALU.mult)

        if _stage in ("pool", "poolng"):
            dbg = pool.tile([8, 238], F32)
            nc.vector.memset(dbg[:], 0.0)
            nc.vector.tensor_copy(out=dbg[:8, :8],
                                  in_=falls[0][:8, 0, :].unsqueeze(1))
            nc.sync.dma_start(out_d[:], dbg[:])
            return nc

        # ------------- head matmuls + AllReduce ---------------------------
        hps = [hpsp.tile([P, B], F32, tag=f"hps{h}", name=f"hps{h}")
               for h in range(7)]
        for chunk in range(24):
            Fj = falls[chunk // 8][:, :, chunk % 8]
            for h in range(7):
                nc.tensor.matmul(
                    out=hps[h][:], lhsT=whp[:, chunk, h * P:(h + 1) * P],
                    rhs=Fj, start=(chunk == 0), stop=(chunk == 23))
        hcat = pool.tile([P, 7 * B], F32)
        for h in range(7):
            nc.any.tensor_copy(out=hcat[:, h * B:(h + 1) * B], in_=hps[h][:])
        cc_in2 = dram.tile([P, 7 * B], F32)
        cc_out2 = dram.tile([P, 7 * B], F32)
        nc.sync.dma_start(cc_in2[:], hcat[:])
        nc.gpsimd.collective_compute(
            "AllReduce", ALU.add, replica_groups=[list(range(NCORES))],
            ins=[cc_in2.opt()], outs=[cc_out2.opt()])
        hsum = pool.tile([P, 7 * B], F32)
        nc.sync.dma_start(hsum[:], cc_out2[:])

        # ------------- relu(h + b1) and the 7 output heads ----------------
        b1t = pool.tile([P, 7], F32)
        nc.sync.dma_start(b1t[:], b1t_d[:])
        w2c = pool.tile([P, 238], F32)
        nc.sync.dma_start(w2c[:], w2c_d[:])
        b2r = pool.tile([8, 238], F32)
        nc.sync.dma_start(b2r[:], b2r_d[:])
        hrelu = pool.tile([P, 7 * B], F32)
        for h in range(7):
            nc.scalar.activation(out=hrelu[:, h * B:(h + 1) * B],
                                 in_=hsum[:, h * B:(h + 1) * B],
                                 func=ACTF.Relu, bias=b1t[:, h:h + 1],
                                 scale=1.0)
        outs_sb = pool.tile([8, 238], F32)
        offs = [0, 38, 63, 98, 133, 168, 203, 238]
        for h in range(7):
            n_h = offs[h + 1] - offs[h]
            pf = sps.tile([8, 512], F32, tag="sps", name=f"pf{h}")
            nc.tensor.matmul(out=pf[:, :n_h],
                             lhsT=hrelu[:, h * B:(h + 1) * B],
                             rhs=w2c[:, offs[h]:offs[h + 1]],
                             start=True, stop=True)
            nc.vector.tensor_add(out=outs_sb[:, offs[h]:offs[h + 1]],
                                 in0=pf[:, :n_h],
                                 in1=b2r[:, offs[h]:offs[h + 1]])
        nc.sync.dma_start(out_d[:], outs_sb[:])
    return nc


# ---------------------------------------------------------------------------
# host side
# ---------------------------------------------------------------------------
_CACHE = {}


def _get_nc():
    if "nc" not in _CACHE:
        nc = bacc.Bacc("TRN2", target_bir_lowering=False, debug=False,
                       num_devices=NCORES)
        _build(nc)
        nc.compile()
        _CACHE["nc"] = nc
    return _CACHE["nc"]


def _fold_weights(inp):
    w2, b2, w3, b3 = (inp["wr2_w2"], inp["wr2_b2"], inp["wr2_w3"],
                      inp["wr2_b3"])
    b1 = inp["wr2_b1"]
    postfix = np.array(
        [[1.0, 0.0, 1.0, 0.0], [0.0, 1.0, 0.0, 1.0],
         [-0.5, 0.0, 0.5, 0.0], [0.0, -0.5, 0.0, 0.5]], np.float32)
    Wf = np.ascontiguousarray((w2 @ w3 @ postfix).astype(np.float32))
    bf = ((((b1 @ w2 + b2) @ w3) + b3) @ postfix).astype(np.float32)
    return Wf, bf


def _shard_inputs(inp):
    inp = {k: np.ascontiguousarray(np.asarray(v)) for k, v in inp.items()}
    xs = {0: inp["x1"][..., 0], 1: inp["x2"][..., 0], 2: inp["x3"][..., 0]}
    Wf, bf = _fold_weights(inp)
    bf_rep = np.ascontiguousarray(np.repeat(bf[:, None], 8, axis=1))
    scl = np.zeros((1, 96), np.float32)
    for lvl, (H, W, TG, RU) in enumerate(LVLS):
        for ci, s in enumerate([W, H, W, H]):
            scl[0, lvl * 32 + ci * 8: lvl * 32 + ci * 8 + 8] = s
    b1t = np.ascontiguousarray(inp["head_b1"].T)         # (128, 7)
    w2c = np.ascontiguousarray(np.concatenate(
        [inp["w2_prov"], inp["w2_alpha"]]
        + [inp["w2_ad"][t] for t in range(5)], axis=1))  # (128, 238)
    b2c = np.concatenate(
        [inp["b2_prov"], inp["b2_alpha"]]
        + [inp["b2_ad"][t] for t in range(5)])           # (238,)
    b2r = np.ascontiguousarray(np.repeat(b2c[None, :], 8, axis=0))

    hw1 = inp["head_w1"].reshape(7, 3, 64, POOL_H, POOL_W, P)
    w1 = inp["wr2_w1"]

    in_maps = []
    for k in range(NCORES):
        m = {"wf": Wf, "bf": bf_rep, "scl": scl, "b1t": b1t, "w2c": w2c,
             "b2r": b2r}
        for li, (H, W, TG, RU) in enumerate(LVLS):
            flat = xs[li][:, CPC * k:CPC * (k + 1)].reshape(-1, W)
            m[f"xs{li}"] = np.ascontiguousarray(
                np.concatenate([flat, np.zeros((PADROWS, W), np.float32)], 0))
        x3 = xs[2][:, CPC * k:CPC * (k + 1)].reshape(B, 8, 8, P)
        m["x3t"] = np.ascontiguousarray(
            x3.transpose(3, 1, 2, 0).reshape(P, 64 * B))
        w1s = w1[CPC * k * 1024:CPC * (k + 1) * 1024].reshape(8, 8, P, 100)
        m["w1p"] = np.ascontiguousarray(
            w1s.transpose(2, 0, 1, 3).reshape(P, 64 * 100))
        whs = hw1[:, :, CPC * k:CPC * (k + 1)]     # (7, 3, 8c, 16i, 8j, 128)
        m["whp"] = np.ascontiguousarray(
            whs.transpose(2, 3, 1, 4, 0, 5).reshape(P, 24 * D_HEADS))
        in_maps.append(m)
    return in_maps


def kernel(**inputs):
    nc = _get_nc()
    in_maps = _shard_inputs(inputs)
    res = run_bass_kernel_spmd(nc, in_maps, core_ids=list(range(NCORES)))
    _CACHE["last_res"] = res
    out = np.asarray(res.results[0]["out_all"], np.float32)
    o1, o2 = out[:, 0:38], out[:, 38:63]
    oad = [out[:, 63 + 35 * t: 63 + 35 * (t + 1)] for t in range(5)]
    return (o1, o2, *oad)


# revision 12
# speedup vs baseline: 1.1209x; 1.1155x over previous
"""Trainium2 Bass kernel for nn_CombinedModel_15977278341388 (nms_detection).

Sharding (8 NeuronCores, memory-bound):
  * wr2 layer-1 matmul (65536x100, 26MB): contraction sharded by channel;
    (100,B) partials AllReduce'd on-device (3.2KB).
  * wr2 layers 2/3 + postfix folded on host into one (100,4) matmul (weight
    preprocessing only -- no input data touched on host).
  * ROI adaptive max pool: channel-sharded (each core pools its 8 channels of
    all samples / all 3 pyramid levels).  Row windows are fetched straight
    from HBM with dma_gather (per-(c,i) row blocks), trailing garbage rows
    masked via tensor_mask_reduce's select-to -FLT_MAX, then a static
    tensor_reduce max; column bins via tensor_mask_reduce windows whose
    per-(b,j) bounds are computed on-device from the predicted boxes.
  * head_w1 (7x24576x128, 88MB): contraction-dim sharded to exactly the
    pooled channels each core owns -> per-core (128hcol, 7*8) partials,
    one 28KB AllReduce, then ReLU + 7 small output matmuls (replicated).

kernel(**inputs) takes FULL inputs, shards internally, returns the FULL
7-tuple (o1, o2, ad0..ad4).
"""
import numpy as np
from contextlib import ExitStack

import concourse.bass as bass
import concourse.mybir as mybir
import concourse.tile as tile
from concourse import bacc
from concourse.bass_utils import run_bass_kernel_spmd

F32 = mybir.dt.float32
I16 = mybir.dt.int16
I32 = mybir.dt.int32
ALU = mybir.AluOpType
ACTF = mybir.ActivationFunctionType
AXX = mybir.AxisListType

P = 128
B = 8
NCORES = 8
CPC = 8            # channels per core
POOL_H, POOL_W = 16, 8
NEGF = float(np.finfo(np.float32).min)

# (H, W, gather_rows, row_unit).  x3 uses 2-row units so the dma_gather
# stride is 256B (the HW minimum); a 4-row window + parity mask covers any
# 3-row bin window.
LVLS = [(128, 128, 9, 1), (64, 64, 5, 1), (32, 32, 4, 2)]
PADROWS = 16
D_HEADS = 7 * 128  # 896


def _floor_inplace(nc, spool, ap, tag):
    """ap <- floor(ap) for non-negative f32 values (|x| < 2^31).

    f32->i32->f32 round trip gives either trunc(x) or rne(x) depending on
    the cast mode; both land in {floor(x), floor(x)+1}, so subtracting the
    (rounded > x) flag yields an exact floor under either semantics.
    """
    ti = spool.tile(list(ap.shape), I32, tag=tag + "i")
    nc.vector.tensor_copy(out=ti[:], in_=ap)
    tf = spool.tile(list(ap.shape), F32, tag=tag + "f")
    nc.vector.tensor_copy(out=tf[:], in_=ti[:])
    gt = spool.tile(list(ap.shape), F32, tag=tag + "g")
    nc.vector.tensor_tensor(out=gt[:], in0=tf[:], in1=ap, op=ALU.is_gt)
    nc.vector.tensor_tensor(out=ap, in0=tf[:], in1=gt[:], op=ALU.subtract)


def _build(nc):
    # ---------------- DRAM I/O ----------------
    xs_dram = [
        nc.dram_tensor(f"xs{li}", [B * CPC * H + PADROWS, W], F32,
                       kind="ExternalInput")
        for li, (H, W, TG, RU) in enumerate(LVLS)
    ]
    x3t_d = nc.dram_tensor("x3t", [P, 64 * B], F32, kind="ExternalInput")
    w1p_d = nc.dram_tensor("w1p", [P, 64 * 100], F32, kind="ExternalInput")
    whp_d = nc.dram_tensor("whp", [P, 24 * D_HEADS], F32, kind="ExternalInput")
    wf_d = nc.dram_tensor("wf", [100, 4], F32, kind="ExternalInput")
    bf_d = nc.dram_tensor("bf", [4, 8], F32, kind="ExternalInput")
    scl_d = nc.dram_tensor("scl", [1, 96], F32, kind="ExternalInput")
    b1t_d = nc.dram_tensor("b1t", [P, 7], F32, kind="ExternalInput")
    w2c_d = nc.dram_tensor("w2c", [P, 238], F32, kind="ExternalInput")
    b2r_d = nc.dram_tensor("b2r", [8, 238], F32, kind="ExternalInput")
    out_d = nc.dram_tensor("out_all", [8, 238], F32, kind="ExternalOutput")

    with tile.TileContext(nc) as tc, ExitStack() as ctx:
        pool = ctx.enter_context(tc.tile_pool(name="main", bufs=1))
        gpool = ctx.enter_context(tc.tile_pool(name="gath", bufs=3))
        spool = ctx.enter_context(tc.tile_pool(name="scr", bufs=2))
        hpsp = ctx.enter_context(tc.tile_pool(name="hps", bufs=1, space="PSUM"))
        sps = ctx.enter_context(tc.tile_pool(name="sps", bufs=1, space="PSUM"))
        dram = ctx.enter_context(tc.tile_pool(name="dram", bufs=1, space="DRAM"))

        # ------------- stage A: h1 partial = x3_shard @ W1_shard ----------
        x3t = pool.tile([P, 64, B], F32)
        nc.sync.dma_start(x3t[:], x3t_d[:].rearrange("p (c b) -> p c b", b=B))
        w1p = pool.tile([P, 64, 100], F32)
        nc.sync.dma_start(w1p[:], w1p_d[:].rearrange("p (c o) -> p c o", o=100))
        # head weights: issued after the small stage-A loads so the PE can
        # start immediately; streams during box math + pooling
        whp = pool.tile([P, 24, D_HEADS], F32)
        for blk in range(8):
            nc.sync.dma_start(
                whp[:, 3 * blk:3 * (blk + 1)],
                whp_d[:, 3 * blk * D_HEADS:3 * (blk + 1) * D_HEADS]
                .rearrange("p (c n) -> p c n", n=D_HEADS))
        h1ps = sps.tile([100, B], F32, tag="sps", name="h1ps")
        for ch in range(64):
            nc.tensor.matmul(out=h1ps[:], lhsT=w1p[:, ch], rhs=x3t[:, ch],
                             start=(ch == 0), stop=(ch == 63))
        h1sb = pool.tile([100, B], F32)
        nc.any.tensor_copy(out=h1sb[:], in_=h1ps[:])

        cc_in1 = dram.tile([100, B], F32)
        cc_out1 = dram.tile([100, B], F32)
        nc.sync.dma_start(cc_in1[:], h1sb[:])
        nc.gpsimd.collective_compute(
            "AllReduce", ALU.add, replica_groups=[list(range(NCORES))],
            ins=[cc_in1.opt()], outs=[cc_out1.opt()])
        h1t = pool.tile([100, B], F32)
        nc.sync.dma_start(h1t[:], cc_out1[:])

        # ------------- boxes: boxT = clip(h1 @ Wf + bf, 0, 1) -------------
        wf = pool.tile([100, 4], F32)
        nc.sync.dma_start(wf[:], wf_d[:])
        bfr = pool.tile([4, 8], F32)
        nc.sync.dma_start(bfr[:], bf_d[:])
        boxps = sps.tile([4, B], F32, tag="sps", name="boxps")
        nc.tensor.matmul(out=boxps[:], lhsT=wf[:], rhs=h1t[:],
                         start=True, stop=True)
        boxT = pool.tile([4, B], F32)
        nc.vector.tensor_add(out=boxT[:], in0=boxps[:], in1=bfr[:])
        nc.vector.tensor_scalar(out=boxT[:], in0=boxT[:], scalar1=0.0,
                                scalar2=1.0, op0=ALU.max, op1=ALU.min)

        # cRow (1,32) = [x1(8) | y1(8) | x2(8) | y2(8)] on partition 0
        cRow = pool.tile([1, 32], F32)
        nc.sync.dma_start(cRow[:].rearrange("p (c b) -> p c b", b=B), boxT[:])

        # ------------- integer coords for the 3 levels --------------------
        scl = pool.tile([1, 96], F32)      # [lvl, coord, b] = W/H/W/H
        nc.sync.dma_start(scl[:], scl_d[:])
        cAll = pool.tile([1, 96], F32)
        nc.vector.tensor_tensor(
            out=cAll[:].rearrange("p (l c) -> p l c", l=3),
            in0=cRow[:].unsqueeze(1).to_broadcast([1, 3, 32]),
            in1=scl[:].rearrange("p (l c) -> p l c", l=3), op=ALU.mult)
        _floor_inplace(nc, spool, cAll[:], "fr96")

        def crd(lvl, c):  # (1, 8) coordinate row of a level
            return cAll[:, lvl * 32 + c * 8: lvl * 32 + c * 8 + 8]

        # lenRow (1,48): [lvl][leny(8) | lenx(8)];  validRow (1,24): [lvl, b]
        lenRow = pool.tile([1, 48], F32)
        validRow = pool.tile([1, 24], F32)
        for lvl, (H, W, TG, RU) in enumerate(LVLS):
            ly = lenRow[:, lvl * 16: lvl * 16 + 8]
            lx = lenRow[:, lvl * 16 + 8: lvl * 16 + 16]
            nc.vector.tensor_tensor(out=ly, in0=crd(lvl, 3), in1=crd(lvl, 1),
                                    op=ALU.subtract)
            nc.vector.tensor_scalar(out=ly, in0=ly, scalar1=1.0, scalar2=1.0,
                                    op0=ALU.add, op1=ALU.max)
            nc.vector.tensor_tensor(out=lx, in0=crd(lvl, 2), in1=crd(lvl, 0),
                                    op=ALU.subtract)
            nc.vector.tensor_scalar(out=lx, in0=lx, scalar1=1.0, scalar2=1.0,
                                    op0=ALU.add, op1=ALU.max)
            vr = validRow[:, lvl * 8: lvl * 8 + 8]
            nc.vector.tensor_scalar(out=vr, in0=crd(lvl, 2), scalar1=float(W),
                                    scalar2=None, op0=ALU.is_lt)
            for in0, in1, op in ((crd(lvl, 3), float(H), ALU.is_lt),
                                 (crd(lvl, 2), crd(lvl, 0), ALU.is_gt),
                                 (crd(lvl, 3), crd(lvl, 1), ALU.is_gt)):
                vv = spool.tile([1, 8], F32, tag="vv")
                if isinstance(in1, float):
                    nc.vector.tensor_scalar(out=vv[:], in0=in0, scalar1=in1,
                                            scalar2=None, op0=op)
                else:
                    nc.vector.tensor_tensor(out=vv[:], in0=in0, in1=in1, op=op)
                nc.vector.tensor_tensor(out=vr, in0=vr, in1=vv[:], op=ALU.mult)

        # ------------- W-stage bounds rows (1, 192) -----------------------
        # col bin j of (lvl, b): [x1c + floor(j*lenx/8),
        #                         x1c + floor((j*lenx + lenx + 7)/8))
        jio = pool.tile([1, 192], I32)
        nc.gpsimd.iota(jio[:], pattern=[[0, 24], [1, 8]], base=0,
                       channel_multiplier=0)
        jiof = pool.tile([1, 192], F32)
        nc.any.tensor_copy(out=jiof[:], in_=jio[:])
        lenxB = pool.tile([1, 192], F32)
        x1cB = pool.tile([1, 192], F32)
        for lvl in range(3):
            nc.any.tensor_copy(
                out=lenxB[:, lvl * 64:(lvl + 1) * 64].rearrange(
                    "p (b j) -> p b j", j=8),
                in_=lenRow[:, lvl * 16 + 8: lvl * 16 + 16]
                .unsqueeze(2).to_broadcast([1, 8, 8]))
            nc.any.tensor_copy(
                out=x1cB[:, lvl * 64:(lvl + 1) * 64].rearrange(
                    "p (b j) -> p b j", j=8),
                in_=crd(lvl, 0).unsqueeze(2).to_broadcast([1, 8, 8]))
        tj = pool.tile([1, 192], F32)
        nc.vector.tensor_tensor(out=tj[:], in0=jiof[:], in1=lenxB[:],
                                op=ALU.mult)
        swRow = pool.tile([1, 192], F32)
        ewRow = pool.tile([1, 192], F32)
        nc.vector.tensor_scalar(out=swRow[:], in0=tj[:], scalar1=0.125,
                                scalar2=None, op0=ALU.mult)
        _floor_inplace(nc, spool, swRow[:], "fr192")
        nc.vector.tensor_tensor(out=ewRow[:], in0=tj[:], in1=lenxB[:],
                                op=ALU.add)
        nc.vector.tensor_scalar(out=ewRow[:], in0=ewRow[:], scalar1=7.0,
                                scalar2=0.125, op0=ALU.add, op1=ALU.mult)
        _floor_inplace(nc, spool, ewRow[:], "fr192")
        nc.vector.tensor_tensor(out=swRow[:], in0=swRow[:], in1=x1cB[:],
                                op=ALU.add)
        nc.vector.tensor_tensor(out=ewRow[:], in0=ewRow[:], in1=x1cB[:],
                                op=ALU.add)
        swB = pool.tile([P, 192], F32)
        ewB = pool.tile([P, 192], F32)
        nc.gpsimd.partition_broadcast(swB[:], swRow[:])
        nc.gpsimd.partition_broadcast(ewB[:], ewRow[:])
        validB = pool.tile([P, 24], F32)
        nc.gpsimd.partition_broadcast(validB[:], validRow[:])

        # ------------- H-stage bins in T-layout (16 partitions = i) -------
        iio = pool.tile([16, 8], I32)
        nc.gpsimd.iota(iio[:], pattern=[[0, 8]], base=0, channel_multiplier=1)
        iiof = pool.tile([16, 8], F32)
        nc.any.tensor_copy(out=iiof[:], in_=iio[:])

        idxF = pool.tile([16, 192], F32)   # gather-unit index, [lvl, b, c]
        maskT = pool.tile([16, 40], F32)   # [mendL0|mendL1|mendL2|mstartL2|-]
        for lvl, (H, W, TG, RU) in enumerate(LVLS):
            lyB = spool.tile([16, 8], F32, tag="lyB")
            nc.gpsimd.partition_broadcast(
                lyB[:], lenRow[:, lvl * 16: lvl * 16 + 8])
            y1B = spool.tile([16, 8], F32, tag="y1B")
            nc.gpsimd.partition_broadcast(y1B[:], crd(lvl, 1))
            ti = spool.tile([16, 8], F32, tag="ti")
            nc.vector.tensor_tensor(out=ti[:], in0=iiof[:], in1=lyB[:],
                                    op=ALU.mult)
            lo = spool.tile([16, 8], F32, tag="lo")
            nc.vector.tensor_scalar(out=lo[:], in0=ti[:], scalar1=0.0625,
                                    scalar2=None, op0=ALU.mult)
            _floor_inplace(nc, spool, lo[:], "fr16")
            hi = spool.tile([16, 8], F32, tag="hi")
            nc.vector.tensor_tensor(out=hi[:], in0=ti[:], in1=lyB[:],
                                    op=ALU.add)
            nc.vector.tensor_scalar(out=hi[:], in0=hi[:], scalar1=15.0,
                                    scalar2=0.0625, op0=ALU.add, op1=ALU.mult)
            _floor_inplace(nc, spool, hi[:], "fr16")
            st = spool.tile([16, 8], F32, tag="st")
            nc.vector.tensor_tensor(out=st[:], in0=y1B[:], in1=lo[:],
                                    op=ALU.add)
            lenT = spool.tile([16, 8], F32, tag="lenT")
            nc.vector.tensor_tensor(out=lenT[:], in0=hi[:], in1=lo[:],
                                    op=ALU.subtract)
            if RU == 1:
                nc.vector.tensor_copy(out=maskT[:, lvl * 8:(lvl + 1) * 8],
                                      in_=lenT[:])
                unit = st
            else:
                half = spool.tile([16, 8], F32, tag="half")
                nc.vector.tensor_scalar(out=half[:], in0=st[:], scalar1=0.5,
                                        scalar2=None, op0=ALU.mult)
                _floor_inplace(nc, spool, half[:], "fr16")
                par = spool.tile([16, 8], F32, tag="par")
                nc.vector.tensor_scalar(out=par[:], in0=half[:], scalar1=-2.0,
                                        scalar2=None, op0=ALU.mult)
                nc.vector.tensor_tensor(out=par[:], in0=par[:], in1=st[:],
                                        op=ALU.add)
                nc.vector.tensor_copy(out=maskT[:, 16:24], in_=par[:])
                nc.vector.tensor_tensor(out=maskT[:, 24:32], in0=par[:],
                                        in1=lenT[:], op=ALU.add)
                unit = half
            # idxF[i, lvl*64 + b*8 + c] = unit(i, b) + (H/RU)*(8*b + c)
            upc = H // RU
            bio = spool.tile([16, 64], I32, tag="bio")
            nc.gpsimd.iota(bio[:], pattern=[[upc * 8, 8], [upc, 8]], base=0,
                           channel_multiplier=0)
            biof = spool.tile([16, 64], F32, tag="biof")
            nc.any.tensor_copy(out=biof[:], in_=bio[:])
            nc.vector.tensor_tensor(
                out=idxF[:, lvl * 64:(lvl + 1) * 64].rearrange(
                    "p (b c) -> p b c", c=8),
                in0=biof[:].rearrange("p (b c) -> p b c", c=8),
                in1=unit[:].unsqueeze(2).to_broadcast([16, 8, 8]),
                op=ALU.add)

        idx16r = pool.tile([16, 192], I16)
        nc.any.tensor_copy(out=idx16r[:], in_=idxF[:])
        idx16 = pool.tile([P, 192], I16)
        maskF = pool.tile([P, 40], F32)
        for g in range(8):
            nc.sync.dma_start(idx16[16 * g:16 * (g + 1), :], idx16r[:])
            nc.sync.dma_start(maskF[16 * g:16 * (g + 1), :], maskT[:])

        import os as _os
        _stage = _os.environ.get("KSTAGE", "full")
        if _stage == "box":
            dbg = pool.tile([8, 238], F32)
            nc.vector.memset(dbg[:], 0.0)
            nc.vector.tensor_copy(out=dbg[:4, :8], in_=boxT[:])
            nc.sync.dma_start(out_d[:], dbg[:])
            return nc

        # ------------- pooling -------------------------------------------
        # Static per-level iotas: t index (for the row mask) and per-j w index
        falls = {}
        for lvl, (H, W, TG, RU) in enumerate(LVLS):
            upc = H // RU
            n_units = B * CPC * upc + (PADROWS - TG) // RU
            in_gather = bass.AP(xs_dram[lvl].ap().tensor, 0,
                                [[W * RU, n_units], [1, TG * W]])
            iti = spool.tile([P, TG], I32, tag="iti")
            nc.gpsimd.iota(iti[:], pattern=[[1, TG]], base=0,
                           channel_multiplier=0)
            itf = pool.tile([P, TG], F32, name=f"itf{lvl}")
            nc.any.tensor_copy(out=itf[:], in_=iti[:])
            iwi = spool.tile([P, POOL_W * W], I32, tag="iwi")
            nc.gpsimd.iota(iwi[:], pattern=[[0, POOL_W], [1, W]], base=0,
                           channel_multiplier=0)
            iwf = pool.tile([P, POOL_W * W], F32, name=f"iwf{lvl}")
            nc.any.tensor_copy(out=iwf[:], in_=iwi[:])
            fall = pool.tile([P, B, POOL_W], F32, name=f"fall{lvl}")
            falls[lvl] = fall
            for b in range(B):
                g = gpool.tile([P, TG * W], F32, tag="g")
                if _stage == "poolng":
                    nc.vector.memset(g[:], 0.0)
                else:
                    nc.gpsimd.dma_gather(
                        out_ap=g[:].unsqueeze(1),
                        in_ap=in_gather,
                        idxs_ap=idx16[:, lvl * 64 + b * 8: lvl * 64 + b * 8 + 8],
                        num_idxs=P, num_idxs_reg=P,
                        elem_size=TG * W, elem_step=W * RU)
                # madd[p, t] = 0 if row t is inside this (b, i=p%16) bin
                madd = spool.tile([P, TG], F32, tag="madd")
                if RU == 1:
                    nc.vector.tensor_scalar(
                        out=madd[:], in0=itf[:],
                        scalar1=maskF[:, lvl * 8 + b: lvl * 8 + b + 1],
                        scalar2=None, op0=ALU.is_lt)
                    nc.vector.tensor_scalar(
                        out=madd[:], in0=madd[:], scalar1=1.0, scalar2=1e30,
                        op0=ALU.subtract, op1=ALU.mult)
                else:
                    m2 = spool.tile([P, TG], F32, tag="m2x3")
                    nc.vector.tensor_scalar(
                        out=madd[:], in0=itf[:],
                        scalar1=maskF[:, 16 + b: 17 + b],
                        scalar2=None, op0=ALU.is_ge)
                    nc.vector.tensor_scalar(
                        out=m2[:], in0=itf[:],
                        scalar1=maskF[:, 24 + b: 25 + b],
                        scalar2=None, op0=ALU.is_lt)
                    nc.vector.tensor_tensor(out=madd[:], in0=madd[:],
                                            in1=m2[:], op=ALU.add)
                    nc.vector.tensor_scalar(
                        out=madd[:], in0=madd[:], scalar1=2.0, scalar2=1e30,
                        op0=ALU.subtract, op1=ALU.mult)
                # H-stage: one masked add over (t, w), then a max chain
                sel = gpool.tile([P, TG * W], F32, tag="sel")
                nc.vector.tensor_tensor(
                    out=sel[:].rearrange("p (t w) -> p t w", w=W),
                    in0=g[:].rearrange("p (t w) -> p t w", w=W),
                    in1=madd[:].unsqueeze(2).to_broadcast([P, TG, W]),
                    op=ALU.add)
                s1h = gpool.tile([P, W], F32, tag="s1h")
                nc.vector.tensor_tensor(out=s1h[:], in0=sel[:, 0:W],
                                        in1=sel[:, W:2 * W], op=ALU.max)
                for t in range(2, TG):
                    nc.vector.tensor_tensor(out=s1h[:], in0=s1h[:],
                                            in1=sel[:, t * W:(t + 1) * W],
                                            op=ALU.max)
                # W-stage: all 8 column bins at once on the (8j, W) grid
                ge = spool.tile([P, POOL_W, W], F32, tag="ge")
                nc.vector.tensor_tensor(
                    out=ge[:],
                    in0=iwf[:].rearrange("p (j w) -> p j w", w=W),
                    in1=swB[:, lvl * 64 + b * 8: lvl * 64 + b * 8 + 8]
                    .unsqueeze(2).to_broadcast([P, POOL_W, W]),
                    op=ALU.is_ge)
                lt = spool.tile([P, POOL_W, W], F32, tag="lt")
                nc.vector.tensor_tensor(
                    out=lt[:],
                    in0=iwf[:].rearrange("p (j w) -> p j w", w=W),
                    in1=ewB[:, lvl * 64 + b * 8: lvl * 64 + b * 8 + 8]
                    .unsqueeze(2).to_broadcast([P, POOL_W, W]),
                    op=ALU.is_lt)
                nc.vector.tensor_tensor(out=ge[:], in0=ge[:], in1=lt[:],
                                        op=ALU.add)
                nc.vector.tensor_scalar(
                    out=ge[:].rearrange("p j w -> p (j w)"),
                    in0=ge[:].rearrange("p j w -> p (j w)"),
                    scalar1=2.0, scalar2=1e30, op0=ALU.subtract, op1=ALU.mult)
                nc.vector.tensor_tensor(
                    out=ge[:], in0=ge[:],
                    in1=s1h[:].unsqueeze(1).to_broadcast([P, POOL_W, W]),
                    op=ALU.add)
                nc.vector.tensor_reduce(out=fall[:, b, :], in_=ge[:],
                                        axis=AXX.X, op=ALU.max)
            nc.vector.tensor_tensor(
                out=fall[:], in0=fall[:],
                in1=validB[:, lvl * 8: lvl * 8 + 8]
                .unsqueeze(2).to_broadcast([P, B, POOL_W]), op=ALU.mult)


        if _stage in ("pool", "poolng"):
            dbg = pool.tile([8, 238], F32)
            nc.vector.memset(dbg[:], 0.0)
            nc.vector.tensor_copy(out=dbg[:8, :8],
                                  in_=falls[0][:8, 0, :].unsqueeze(1))
            nc.sync.dma_start(out_d[:], dbg[:])
            return nc

        # ------------- head matmuls + AllReduce ---------------------------
        hps = [hpsp.tile([P, B], F32, tag=f"hps{h}", name=f"hps{h}")
               for h in range(7)]
        for chunk in range(24):
            Fj = falls[chunk // 8][:, :, chunk % 8]
            for h in range(7):
                nc.tensor.matmul(
                    out=hps[h][:], lhsT=whp[:, chunk, h * P:(h + 1) * P],
                    rhs=Fj, start=(chunk == 0), stop=(chunk == 23))
        hcat = pool.tile([P, 7 * B], F32)
        for h in range(7):
            nc.any.tensor_copy(out=hcat[:, h * B:(h + 1) * B], in_=hps[h][:])
        cc_in2 = dram.tile([P, 7 * B], F32)
        cc_out2 = dram.tile([P, 7 * B], F32)
        nc.sync.dma_start(cc_in2[:], hcat[:])
        nc.gpsimd.collective_compute(
            "AllReduce", ALU.add, replica_groups=[list(range(NCORES))],
            ins=[cc_in2.opt()], outs=[cc_out2.opt()])
        hsum = pool.tile([P, 7 * B], F32)
        nc.sync.dma_start(hsum[:], cc_out2[:])

        # ------------- relu(h + b1) and the 7 output heads ----------------
        b1t = pool.tile([P, 7], F32)
        nc.sync.dma_start(b1t[:], b1t_d[:])
        w2c = pool.tile([P, 238], F32)
        nc.sync.dma_start(w2c[:], w2c_d[:])
        b2r = pool.tile([8, 238], F32)
        nc.sync.dma_start(b2r[:], b2r_d[:])
        hrelu = pool.tile([P, 7 * B], F32)
        for h in range(7):
            nc.scalar.activation(out=hrelu[:, h * B:(h + 1) * B],
                                 in_=hsum[:, h * B:(h + 1) * B],
                                 func=ACTF.Relu, bias=b1t[:, h:h + 1],
                                 scale=1.0)
        outs_sb = pool.tile([8, 238], F32)
        offs = [0, 38, 63, 98, 133, 168, 203, 238]
        for h in range(7):
            n_h = offs[h + 1] - offs[h]
            pf = sps.tile([8, 512], F32, tag="sps", name=f"pf{h}")
            nc.tensor.matmul(out=pf[:, :n_h],
                             lhsT=hrelu[:, h * B:(h + 1) * B],
                             rhs=w2c[:, offs[h]:offs[h + 1]],
                             start=True, stop=True)
            nc.vector.tensor_add(out=outs_sb[:, offs[h]:offs[h + 1]],
                                 in0=pf[:, :n_h],
                                 in1=b2r[:, offs[h]:offs[h + 1]])
        nc.sync.dma_start(out_d[:], outs_sb[:])
    return nc


# ---------------------------------------------------------------------------
# host side
# ---------------------------------------------------------------------------
_CACHE = {}


def _get_nc():
    if "nc" not in _CACHE:
        nc = bacc.Bacc("TRN2", target_bir_lowering=False, debug=False,
                       num_devices=NCORES)
        _build(nc)
        nc.compile()
        _CACHE["nc"] = nc
    return _CACHE["nc"]


def _fold_weights(inp):
    w2, b2, w3, b3 = (inp["wr2_w2"], inp["wr2_b2"], inp["wr2_w3"],
                      inp["wr2_b3"])
    b1 = inp["wr2_b1"]
    postfix = np.array(
        [[1.0, 0.0, 1.0, 0.0], [0.0, 1.0, 0.0, 1.0],
         [-0.5, 0.0, 0.5, 0.0], [0.0, -0.5, 0.0, 0.5]], np.float32)
    Wf = np.ascontiguousarray((w2 @ w3 @ postfix).astype(np.float32))
    bf = ((((b1 @ w2 + b2) @ w3) + b3) @ postfix).astype(np.float32)
    return Wf, bf


def _shard_inputs(inp):
    inp = {k: np.ascontiguousarray(np.asarray(v)) for k, v in inp.items()}
    xs = {0: inp["x1"][..., 0], 1: inp["x2"][..., 0], 2: inp["x3"][..., 0]}
    Wf, bf = _fold_weights(inp)
    bf_rep = np.ascontiguousarray(np.repeat(bf[:, None], 8, axis=1))
    scl = np.zeros((1, 96), np.float32)
    for lvl, (H, W, TG, RU) in enumerate(LVLS):
        for ci, s in enumerate([W, H, W, H]):
            scl[0, lvl * 32 + ci * 8: lvl * 32 + ci * 8 + 8] = s
    b1t = np.ascontiguousarray(inp["head_b1"].T)         # (128, 7)
    w2c = np.ascontiguousarray(np.concatenate(
        [inp["w2_prov"], inp["w2_alpha"]]
        + [inp["w2_ad"][t] for t in range(5)], axis=1))  # (128, 238)
    b2c = np.concatenate(
        [inp["b2_prov"], inp["b2_alpha"]]
        + [inp["b2_ad"][t] for t in range(5)])           # (238,)
    b2r = np.ascontiguousarray(np.repeat(b2c[None, :], 8, axis=0))

    hw1 = inp["head_w1"].reshape(7, 3, 64, POOL_H, POOL_W, P)
    w1 = inp["wr2_w1"]

    in_maps = []
    for k in range(NCORES):
        m = {"wf": Wf, "bf": bf_rep, "scl": scl, "b1t": b1t, "w2c": w2c,
             "b2r": b2r}
        for li, (H, W, TG, RU) in enumerate(LVLS):
            flat = xs[li][:, CPC * k:CPC * (k + 1)].reshape(-1, W)
            m[f"xs{li}"] = np.ascontiguousarray(
                np.concatenate([flat, np.zeros((PADROWS, W), np.float32)], 0))
        x3 = xs[2][:, CPC * k:CPC * (k + 1)].reshape(B, 8, 8, P)
        m["x3t"] = np.ascontiguousarray(
            x3.transpose(3, 1, 2, 0).reshape(P, 64 * B))
        w1s = w1[CPC * k * 1024:CPC * (k + 1) * 1024].reshape(8, 8, P, 100)
        m["w1p"] = np.ascontiguousarray(
            w1s.transpose(2, 0, 1, 3).reshape(P, 64 * 100))
        whs = hw1[:, :, CPC * k:CPC * (k + 1)]     # (7, 3, 8c, 16i, 8j, 128)
        m["whp"] = np.ascontiguousarray(
            whs.transpose(2, 3, 1, 4, 0, 5).reshape(P, 24 * D_HEADS))
        in_maps.append(m)
    return in_maps


def kernel(**inputs):
    nc = _get_nc()
    in_maps = _shard_inputs(inputs)
    res = run_bass_kernel_spmd(nc, in_maps, core_ids=list(range(NCORES)))
    _CACHE["last_res"] = res
    out = np.asarray(res.results[0]["out_all"], np.float32)
    o1, o2 = out[:, 0:38], out[:, 38:63]
    oad = [out[:, 63 + 35 * t: 63 + 35 * (t + 1)] for t in range(5)]
    return (o1, o2, *oad)
